# revision 46
# baseline (speedup 1.0000x reference)
"""Multi-head self-attention Trainium2 Bass kernel.

Problem: B=4, T=2048, EMB=1024, H=16 heads (head_dim 64), causal mask,
scores scaled by 1/sqrt(EMB), torch-Linear style projections.

Sharding (8 cores): data-parallel over the 4 batches x tensor-parallel over
2 head-groups of 8 heads.  Core c handles batch c//2, heads (c%2)*8..+8.
Each core computes q/k/v projections for its head shard, full TxT causal
attention for its 8 heads, and a partial output projection (its 512 rows of
the unify matmul).  Host sums the two partial outputs per batch and adds bo.

Device layout notes:
 - All PE operands are fp16 (1 col/cycle on the PE); PSUM accumulates fp32.
 - x and the weights are pre-transposed/cast on host so that every matmul
   contraction dim lands on the SBUF partition axis.
 - Scores are computed transposed (pT[s, t] = exp(q.k/32)) so that the
   attn @ v contraction (over s) needs no on-device transposes.  The two
   heads of a head-pair run as adjacent matmuls on disjoint PE row groups
   (contraction rows 0-63 / 64-127) writing the two banks of one [128,1024]
   PSUM tile, so they stream concurrently.
 - Causal column restriction: for the diagonal s-chunk at offset di the
   score/exp/AV work only covers query columns >= 128*di; only the leading
   [128,128] triangle block needs a mask multiply.
 - Softmax sums ride as a 65th "ones" column of v; normalization is a
   fast-approx reciprocal + gpsimd partition broadcast + in-place multiply.
 - Scheduling: the PE executes strictly in-order and its clock ramps only
   under continuous activity (gaps reset the ramp), so (a) x loads are
   split by column-block so block-0 projections start after ~2 MB of DMA,
   (b) warm-up matmuls are strung on DMA arrivals in arrival order, (c)
   each block's pair-2/3 q/k projections and the previous blocks' output
   projections are paced into the ACT-heavy chunk loops as fillers, and
   (d) each pair's final AV + softmax-normalization chain is carried past
   the next pair's first score/exp so the PE never drains at pair/block
   boundaries.
 - The output is stored f16 (halves store traffic); the host upcasts and
   sums the two head-group partials in f32.
"""

import numpy as np

B, T, EMB, H = 4, 2048, 1024, 16
HD = 64          # head dim
HPC = 8          # heads per core
DPC = HPC * HD   # projected dim per core = 512
NCORES = 8
E8 = EMB // 128  # contraction chunks over emb = 8
DP = DPC // 128  # head-pair chunks = 4
TB = T // 512    # t-blocks = 4
SC = T // 128    # s-chunks = 16
VW = HD + 1      # v columns per head incl. ones column = 65

_CACHED_NC = None
LAST_RESULTS = None  # BassKernelResults of the most recent run (for test.py)


def _build_nc():
    import concourse.bacc as bacc
    import concourse.tile as tile
    import concourse.mybir as mybir

    f16 = mybir.dt.float16
    f32 = mybir.dt.float32
    Exp = mybir.ActivationFunctionType.Exp

    nc = bacc.Bacc(
        "TRN2",
        target_bir_lowering=False,
        debug=False,
        enable_asserts=False,
        num_devices=NCORES,
    )

    xT_d = nc.dram_tensor("xT", [EMB, T], f16, kind="ExternalInput").ap()
    wqT_d = nc.dram_tensor("wqT", [EMB, DPC], f16, kind="ExternalInput").ap()
    wkT_d = nc.dram_tensor("wkT", [EMB, DPC], f16, kind="ExternalInput").ap()
    wvT_d = nc.dram_tensor("wvT", [EMB, DPC], f16, kind="ExternalInput").ap()
    woT_d = nc.dram_tensor("woT", [DPC, EMB], f16, kind="ExternalInput").ap()
    cm_d = nc.dram_tensor("cmask", [128, 256], f16, kind="ExternalInput").ap()
    # f16 output: halves the store traffic and the final-DMA tail; the host
    # upcasts and sums the two head-group partials in f32.  The f16
    # rounding adds ~5e-4 relative error against a 2e-2 budget.
    out_d = nc.dram_tensor("out", [T, EMB], f16, kind="ExternalOutput").ap()

    with tile.TileContext(nc) as tc:
        # ---- persistent SBUF tensors (static allocations) -------------
        def sb(name, shape, dt=f16):
            return nc.alloc_sbuf_tensor(name, list(shape), dt).ap()

        xt = [sb(f"xt{k}", [128, T]) for k in range(E8)]
        wq = [sb(f"wq{k}", [128, DPC]) for k in range(E8)]
        wk = [sb(f"wk{k}", [128, DPC]) for k in range(E8)]
        wv = [sb(f"wv{k}", [128, DPC]) for k in range(E8)]
        wo = [sb(f"wo{p}", [128, EMB]) for p in range(DP)]
        cm = sb("cm", [128, 256])
        qt = [sb(f"qt{p}", [128, T]) for p in range(DP)]
        kt = [sb(f"kt{p}", [128, T]) for p in range(DP)]
        vt = sb("vt", [128, SC * HPC * VW])
        ytn = [sb(f"ytn{p}", [128, T]) for p in range(DP)]
        # per-head reciprocal softmax sums for the current t-block, packed
        # on partition 0 (custom DVE ops and partition_broadcast want
        # partition-0-based APs); reused across blocks
        rec = sb("rec", [1, HPC * 512], f32)
        sums = sb("sums", [1, HPC * 512], f32)
        # never initialized: warm-up matmuls read garbage (discarded), so
        # they have no dependencies and can start immediately
        warmsrc = sb("warmsrc", [128, 512])
        # block-3 outproj partials (p=0..2), evacuated here so the final
        # tail only runs the p=3 matmul + fused add+store
        opscr = sb("opscr", [128, 8 * 512], f32)
        # fence targets: tiny SBUF->SBUF DMAs reading the last phase-1 tile
        # keep the phase-2 loads out of the DMA rings until phase-1 lands
        # (the rings fair-share bandwidth among everything in flight, so
        # un-fenced phase-2 stretches the phase-1 ramp ~2x)
        fsc = sb("fsc", [1, 8])

        with (
            tc.tile_pool(name="pp", bufs=2, space="PSUM") as pp,
            tc.tile_pool(name="scp", bufs=2, space="PSUM") as scp,
            tc.tile_pool(name="ytp", bufs=1, space="PSUM") as ytp,
            tc.tile_pool(name="ptp", bufs=8) as ptp,
            tc.tile_pool(name="brecp", bufs=4) as brecp,
            tc.tile_pool(name="ost", bufs=3) as ost,
        ):
            # ---- input loads (direct DMA, alternating between the two
            # HWDGE queue engines; Bacc legalizes multi-dep matmul waits) -
            load_rr = [0]

            def load(dst, src):
                eng = nc.sync if load_rr[0] % 2 == 0 else nc.scalar
                load_rr[0] += 1
                eng.dma_start(dst, src)

            # ones columns for the softmax-sum trick: memset only the 65th
            # column of each head block (the v columns get overwritten by
            # the projection evacuations anyway)
            ones3 = vt[:, :].rearrange("p (x c) -> p x c", c=VW)[:, :, HD:VW]
            nc.vector.memset(ones3, 1.0)
            # PE warm-up: keeps the HAM activity window busy through the
            # whole DMA ramp so real matmuls start at the 2.4 GHz clock.
            # The first burst reads uninitialized SBUF (no deps, starts
            # immediately); later warm-ups read each freshly-DMA'd xt
            # chunk, which strings them out across the load timeline.
            warm = pp.tile([128, 512], f32, tag="pp", name="warmup")

            def warmup_burst(src, n):
                for _ in range(n):
                    nc.tensor.matmul(warm[:, :], src[0:128, 0:128],
                                     src[0:128, 0:512], start=True, stop=True)

            warmup_burst(warmsrc, 8)
            # cm is tiny and gates the first diagonal mask multiply in block
            # 0 -- load it before the big tensors so AV(ck=0) never stalls
            load(cm[:, :], cm_d[:, :])
            # Phase-1 loads: only what the block-0 projections contract over
            # (x columns 0:512 = 1 MB instead of the full 4 MB) plus wq/wv/
            # wk, so block-0 attention starts ~15 us earlier.  The tracker
            # keys dependencies on byte ranges, so consumers of the first
            # 512 columns don't wait for the phase-2 column loads.
            for k in range(E8):
                r = slice(k * 128, (k + 1) * 128)
                load(xt[k][:, 0:512], xT_d[r, 0:512])
                load(wq[k][:, :], wqT_d[r, :])
                warmup_burst(xt[k], 2)
            # wv before wk: the pre units run q0, v0, v1, k0 so the two
            # v-chains amortize the later wk arrival
            for k in range(E8):
                r = slice(k * 128, (k + 1) * 128)
                load(wv[k][:, :], wvT_d[r, :])
            for k in range(E8):
                r = slice(k * 128, (k + 1) * 128)
                load(wk[k][:, :], wkT_d[r, :])
            # DMA fence + ALL phase-2 loads on nc.sync (SP) ONLY.  The
            # fence's RAW wait on wk[7] (the last phase-1 load) stalls the
            # SP sequencer so the phase-2 configs behind it are held back
            # and their transfers don't contend with phase-1 for ring
            # bandwidth.  Critically, nothing phase-2 goes on nc.scalar:
            # that is the Activation engine's sequencer, and a fence or a
            # burst of DMA configs there blocks block-0's exps behind it
            # (the earlier dual-engine second fence measured worse for
            # exactly this reason).  SP has no other duty until the output
            # stores at ~45 us.
            nc.sync.dma_start(fsc[0:1, 0:2], wk[E8 - 1][0:1, 0:2])
            # Phase-2: the remaining x columns (blocks 1-3 projections, run
            # as fillers inside blocks 0-2) and wo (first read ~40 us in).
            # Block-1 columns first so block 0's early(1) fillers (consumed
            # from ~16 us) don't stall on the block-2/3 columns behind them.
            for k in range(E8):
                r = slice(k * 128, (k + 1) * 128)
                nc.sync.dma_start(xt[k][:, 512:1024], xT_d[r, 512:1024])
            for k in range(E8):
                r = slice(k * 128, (k + 1) * 128)
                nc.sync.dma_start(xt[k][:, 1024:T], xT_d[r, 1024:T])
            for p in range(DP):
                nc.sync.dma_start(wo[p][:, :],
                                  woT_d[p * 128:(p + 1) * 128, :])

            def project_tiles(j):
                """Per-tile emit closures for block j's q/k/v projections,
                split into (early, late): early units (q0,q1,k0,k1,v0-3) are
                needed by the block's first head-pairs; late units (q2,k2,
                q3,k3) are only read by pairs 2-3 and are deferred into the
                block's own chunk loop as fillers.  (Evac on DVE -- ACT does
                nothing but exp; gpsimd can't read PSUM.)"""
                tcols = slice(j * 512, (j + 1) * 512)
                units = []

                def qk_unit(w_t, dst, p):
                    def emit():
                        dcols = slice(p * 128, (p + 1) * 128)
                        ps = pp.tile([128, 512], f32, tag="pp",
                                     name=f"ps_{p}_{j}")
                        for e in range(E8):
                            nc.tensor.matmul(
                                ps[:, :], w_t[e][:, dcols], xt[e][:, tcols],
                                start=(e == 0), stop=(e == E8 - 1),
                            )
                        nc.vector.tensor_copy(dst[p][:, tcols], ps[:, :])
                    return emit

                def v_unit(s):
                    def emit():
                        ps = pp.tile([128, 512], f32, tag="pp",
                                     name=f"psv_{s}")
                        for e in range(E8):
                            nc.tensor.matmul(
                                ps[:, :], xt[e][:, s * 128:(s + 1) * 128],
                                wv[e][:, :],
                                start=(e == 0), stop=(e == E8 - 1),
                            )
                        dst = vt[:, s * HPC * VW:(s + 1) * HPC * VW]
                        dst = dst.rearrange("p (h c) -> p h c",
                                            c=VW)[:, :, 0:HD]
                        src = ps[:, :].rearrange("p (h c) -> p h c", c=HD)
                        nc.vector.tensor_copy(dst, src)
                    return emit

                # early: q0,q1 (DMA delivers wq first), then k0,k1, then v
                # chunks; late: pairs 2-3, deferred into the block itself
                units.append(qk_unit(wq, qt, 0))
                units.append(qk_unit(wq, qt, 1))
                units.append(qk_unit(wk, kt, 0))
                units.append(qk_unit(wk, kt, 1))
                for s in range(4 * j, 4 * j + 4):
                    units.append(v_unit(s))
                late = []
                for p in range(2, DP):
                    late.append(qk_unit(wq, qt, p))
                    late.append(qk_unit(wk, kt, p))
                return units, late

            def outproj_tiles(j):
                """Per-tile emit closures for block j's output projection."""
                units = []

                def op_unit(tcn, n):
                    def emit():
                        trows = slice(tcn * 128, (tcn + 1) * 128)
                        ncols = slice(n * 512, (n + 1) * 512)
                        ps = pp.tile([128, 512], f32, tag="pp",
                                     name=f"op_{tcn}_{n}")
                        for p in range(DP):
                            nc.tensor.matmul(
                                ps[:, :], ytn[p][:, trows], wo[p][:, ncols],
                                start=(p == 0), stop=(p == DP - 1),
                            )
                        ot = ost.tile([128, 512], f16, tag="ot",
                                      name=f"ot_{tcn}_{n}")
                        nc.vector.tensor_copy(ot[:, :], ps[:, :])
                        nc.sync.dma_start(out_d[trows, ncols], ot[:, :])
                    return emit

                for tcn in range(4 * j, 4 * j + 4):
                    for n in range(EMB // 512):
                        units.append(op_unit(tcn, n))
                return units

            def outproj_partial_tiles(j):
                """Pass A of block j's output projection: accumulate the
                p=0..2 partials and park them in SBUF scratch.  Emitted as
                tail fillers inside block j's LAST pair (finalize(p=2) is
                carried into that pair's ck1, so these must not emit
                earlier), where the chunk loop is exp-bound and the PE
                would otherwise starve at the throttled late-kernel
                clock."""
                units = []

                def pa_unit(idx, tcn, n):
                    def emit():
                        trows = slice(tcn * 128, (tcn + 1) * 128)
                        ncols = slice(n * 512, (n + 1) * 512)
                        ps = pp.tile([128, 512], f32, tag="pp",
                                     name=f"opa_{tcn}_{n}")
                        for p in range(DP - 1):
                            nc.tensor.matmul(
                                ps[:, :], ytn[p][:, trows], wo[p][:, ncols],
                                start=(p == 0), stop=(p == DP - 2),
                            )
                        nc.vector.tensor_copy(
                            opscr[:, idx * 512:(idx + 1) * 512], ps[:, :])
                    return emit

                idx = 0
                for tcn in range(4 * j, 4 * j + 4):
                    for n in range(EMB // 512):
                        units.append(pa_unit(idx, tcn, n))
                        idx += 1
                return units

            def outproj_close_tiles(j):
                """Pass B (the true tail): only the p=3 matmul, then a
                fused add-partials + f16-cast evacuation and the store."""
                units = []

                def pb_unit(idx, tcn, n):
                    def emit():
                        trows = slice(tcn * 128, (tcn + 1) * 128)
                        ncols = slice(n * 512, (n + 1) * 512)
                        ps = pp.tile([128, 512], f32, tag="pp",
                                     name=f"opb_{tcn}_{n}")
                        nc.tensor.matmul(
                            ps[:, :], ytn[DP - 1][:, trows],
                            wo[DP - 1][:, ncols], start=True, stop=True)
                        ot = ost.tile([128, 512], f16, tag="ot",
                                      name=f"otb_{tcn}_{n}")
                        nc.vector.tensor_add(
                            ot[:, :], ps[:, :],
                            opscr[:, idx * 512:(idx + 1) * 512])
                        nc.sync.dma_start(out_d[trows, ncols], ot[:, :])
                    return emit

                idx = 0
                for tcn in range(4 * j, 4 * j + 4):
                    for n in range(EMB // 512):
                        units.append(pb_unit(idx, tcn, n))
                        idx += 1
                return units

            # carry[0] defers each pair's final AV matmuls (which wait on
            # the pair's last exp+mask) plus its softmax-normalization
            # chain past the NEXT pair's first score/exp emission, so the
            # next pair's PE work hides the exp latency instead of the PE
            # draining at every pair/block boundary (a drain also resets
            # the PE p-state ramp, compounding the cost).
            carry = [None]

            def attend_block(j, fillers, tail_fillers=(), bridge=False):
                """Attention for t-block j.  `fillers` are independent PE
                tile units (next block's projections, previous block's
                output projection) emitted interleaved into the chunk loop
                so the PE always has queued work while ACT runs the exps.
                `tail_fillers` are units that must not emit before the
                last pair's ck1 (they read every pair's normalized ytn);
                they pace over the last pair's remaining chunks."""
                tcols = slice(j * 512, (j + 1) * 512)
                nchunks = 4 * j + 4
                nunits = DP * nchunks
                fill_acc = [0.0]
                fill_step = len(fillers) / nunits
                fillers = list(fillers)
                tail_fillers = list(tail_fillers)
                tail_acc = [0.0]
                tail_step = (len(tail_fillers) / max(1, nchunks - 2)
                             if tail_fillers else 0.0)

                def emit_fill():
                    fill_acc[0] += fill_step
                    while fillers and fill_acc[0] >= 1.0:
                        fillers.pop(0)()
                        fill_acc[0] -= 1.0

                for p in range(DP):
                    yts = [
                        ytp.tile([VW, 512], f32, tag=f"yt{h2}",
                                 name=f"yt{h2}_{p}_{j}")
                        for h2 in range(2)
                    ]
                    # software-pipelined chunk loop: AV for chunk ck is
                    # emitted after the scores+exp of chunk ck+1 so the PE
                    # always has score work while ACT runs the exp.
                    pends = []  # queue of (ck, pt, off, di, last), depth 2

                    def emit_av(ck, pt, off, di, last, yts=yts, p=p):
                        # yts/p bound by default args: the carried call runs
                        # inside the NEXT pair's scope where the loop
                        # variables have been rebound.
                        # One matmul per head covering [off:512]: the mask
                        # multiply of chunk ck finishes on DVE during
                        # exp(ck+1) on ACT, before this AV executes, so no
                        # clean/masked split is needed.  PSUM has_written
                        # bits make the accumulation order safe: the first
                        # executed matmul of the round clears the bank.
                        first = (ck == 0)
                        for h2 in range(2):
                            h = 2 * p + h2
                            vs = vt[:, ck * HPC * VW + h * VW:
                                    ck * HPC * VW + (h + 1) * VW]
                            nc.tensor.matmul(
                                yts[h2][:, off:],
                                vs, pt[:, h2 * 512 + off:(h2 + 1) * 512],
                                start=first, stop=last,
                            )

                    def finalize(yts=yts, p=p):
                        # per h2: sums-copy + recip (gates the broadcast
                        # chain), then the bulk evacuation -- finishing h2=0
                        # entirely before touching h2=1 releases its PSUM
                        # bank sooner
                        for h2 in range(2):
                            base = h2 * 64
                            r = 2 * p + h2
                            rcols = slice(r * 512, (r + 1) * 512)
                            nc.vector.tensor_copy(sums[0:1, rcols],
                                                  yts[h2][HD:HD + 1, :])
                            nc.vector.reciprocal_approx_fast(
                                rec[0:1, rcols], sums[0:1, rcols])
                            nc.vector.tensor_copy(
                                ytn[p][base:base + 64, tcols],
                                yts[h2][0:HD, :])
                        # deferred normalization for this (p, block)
                        for h2 in range(2):
                            base = h2 * 64
                            r = 2 * p + h2
                            rcols = slice(r * 512, (r + 1) * 512)
                            brec = brecp.tile([128, 512], f32, tag="brec",
                                              name=f"brec{h2}_{p}_{j}")
                            nc.gpsimd.partition_broadcast(brec[:, :],
                                                          rec[0:1, rcols])
                            nc.vector.tensor_mul(
                                ytn[p][base:base + 64, tcols],
                                ytn[p][base:base + 64, tcols],
                                brec[base:base + 64, :],
                            )

                    for ck in range(nchunks):
                        di = ck - 4 * j
                        off = 128 * di if di > 0 else 0
                        sc = scp.tile([128, 1024], f32, tag="sc",
                                      name=f"sc_{p}_{j}_{ck}")
                        pt = ptp.tile([128, 1024], f16, tag="pt",
                                      name=f"pt_{p}_{j}_{ck}")
                        # the two heads on disjoint PE row groups (0-63 /
                        # 64-127) target the two banks of sc -> concurrent
                        for h2 in range(2):
                            base = h2 * 64
                            nc.tensor.matmul(
                                sc[:, h2 * 512 + off:(h2 + 1) * 512],
                                kt[p][base:base + 64,
                                      ck * 128:(ck + 1) * 128],
                                qt[p][base:base + 64,
                                      j * 512 + off:(j + 1) * 512],
                                start=True, stop=True,
                            )
                        # one exp for both heads (strided 3D AP over the
                        # valid column ranges)
                        sc3 = sc[:, :].rearrange("p (h c) -> p h c",
                                                 c=512)[:, :, off:]
                        pt3 = pt[:, :].rearrange("p (h c) -> p h c",
                                                 c=512)[:, :, off:]
                        nc.scalar.activation(pt3, sc3, Exp, scale=1.0 / 32.0)
                        if di >= 0:
                            # triangle mask on the leading 128 columns of
                            # both heads' valid ranges.  Two flat 2D
                            # multiplies instead of one strided 3D AP: all-
                            # SBUF packed 16-bit operands qualify for the
                            # DVE 4x perf mode, which the 3D form does not.
                            # NOTE: must stay on DVE -- gpsimd executes this
                            # multiply ~10x slower (measured 505 us vs 298
                            # us whole-kernel)
                            for h2 in range(2):
                                c0 = h2 * 512 + off
                                nc.vector.tensor_mul(
                                    pt[:, c0:c0 + 128], pt[:, c0:c0 + 128],
                                    cm[:, 0:128])
                        # NOTE: a 2-deep AV pipeline (AV(ck-2) after
                        # exp(ck)) measured worse -- the 2-buf sc pool just
                        # moves the stall from AV to scores(ck+2), and the
                        # reshuffled transitions added ~2 us of gaps
                        if pends:
                            if carry[0] is not None:
                                # previous pair's deferred final AV +
                                # softmax normalization: flushed at ck1,
                                # after two of this pair's score/exp
                                # emissions, maximizing cover of the
                                # previous pair's last exp+mask latency
                                # (must precede this pair's first AV, which
                                # re-opens the aliased yts accumulators)
                                carry[0]()
                                carry[0] = None
                            emit_av(*pends.pop(0))
                        emit_fill()
                        if p == DP - 1 and ck >= 2 and tail_fillers:
                            tail_acc[0] += tail_step
                            while tail_fillers and tail_acc[0] >= 1.0 - 1e-9:
                                tail_fillers.pop(0)()
                                tail_acc[0] -= 1.0
                        pends.append((ck, pt, off, di, ck == nchunks - 1))

                    # one AV still pending: carry it (and this pair's
                    # normalization) past the next pair's first score/exps
                    last_pt = pends[0][1]

                    def make_carry(emit_av=emit_av, pend=pends[0],
                                   fin=finalize):
                        def c():
                            emit_av(*pend)
                            fin()
                        return c
                    carry[0] = make_carry()
                    if p == DP - 1:
                        # flush any fillers left over from float pacing --
                        # fill_step accumulation can round below len(fillers)
                        while fillers:
                            fillers.pop(0)()
                        while tail_fillers:
                            tail_fillers.pop(0)()
                    if bridge and p == DP - 1:
                        carry[0]()
                        carry[0] = None
                        # dummy matmuls reading the final pt tile bridge the
                        # last normalization chain so HAM stays at full
                        # clock for the final output projection; the pt
                        # dependency stops the static scheduler from
                        # hoisting them earlier
                        lpt = last_pt
                        warm2 = pp.tile([128, 512], f32, tag="pp",
                                        name=f"wbr_{j}")
                        for _ in range(22):
                            nc.tensor.matmul(warm2[:, :], lpt[0:128, 0:128],
                                             lpt[0:128, 0:512],
                                             start=True, stop=True)

            # Filler rebalance: block 3 has the most exp work (16 of 40
            # s-chunks) but, in the naive schedule, the fewest fillers, so
            # it runs ACT-bound while blocks 0-2 are PE-bound.  Defer each
            # block's pair-2/3 q/k projections into its own chunk loop
            # (late units, ordered first so they finish before pair 2
            # starts) and push outproj(1) from block 2 into block 3.
            early = {}
            late = {}
            for j in range(TB):
                early[j], late[j] = project_tiles(j)
            # Minimal serial pre-phase: q0, v0, v1, k0 (matching the DMA
            # arrival order wq -> wv -> wk); q1/v2/v3/k1 move into block 0's
            # fillers, paced one per chunk-unit so each lands just before
            # its first consumer (k1 by pair 1, v2/v3 by AV chunks 2/3).
            # Warm-ups between the pre units, gated on progressively later
            # DMA arrivals (wv lands during q0, wk during v0/v1), so
            # nothing later-gated ever queues ahead of ready work on the
            # in-order PE
            e0 = early[0]
            e0[0]()                       # q0 (needs xt cols 0:512 + wq)
            for k in range(E8):
                warmup_burst(wv[k], 1)
            e0[4]()                       # v0 (needs wv)
            e0[5]()                       # v1
            e0[2]()                       # k0 (needs wk)
            block_fill = {
                0: [e0[1], e0[6], e0[7], e0[3]] + late[0] + early[1],
                1: late[1] + early[2] + outproj_tiles(0),
                2: late[2] + early[3],
                3: late[3] + outproj_tiles(1) + outproj_tiles(2),
            }
            # NOTE: outproj_partial/close_tiles (split tail) measured
            # structurally WORSE: block 3 is PE-bound at the throttled
            # late-kernel clock, so the pass-A fillers extend it 1:1 while
            # the DVE-serialized pass-B tail adds ~2 us.  Keep the plain
            # 4-matmul tail.
            for j in range(TB):
                attend_block(j, block_fill[j], bridge=(j == TB - 1))
            for u in outproj_tiles(TB - 1):
                u()

    nc.compile()
    return nc


def _causal_mask_tiles() -> np.ndarray:
    """[128, 256] fp16: the 128x128 causal triangle (keep key p <= query c)
    stored twice side by side so a [128, 2, 128] strided AP lines up with
    the two heads' column blocks of a pt tile."""
    p = np.arange(128)[:, None]
    c = np.arange(128)[None, :]
    tri = (p <= c).astype(np.float16)
    return np.ascontiguousarray(np.concatenate([tri, tri], axis=1))


def _numpy_fallback(x, mask, Wq, bq, Wk, bk, Wv, bv, Wo, bo):
    b, t, emb = x.shape
    h = H
    k = emb // h
    q = (x @ Wq.T + bq).reshape(b, t, h, k)
    kk = (x @ Wk.T + bk).reshape(b, t, h, k)
    v = (x @ Wv.T + bv).reshape(b, t, h, k)
    scale = 1.0 / np.sqrt(emb)
    out = np.empty((b, t, emb), dtype=np.float32)
    for bi in range(b):
        yb = np.empty((t, h, k), dtype=np.float32)
        for hi in range(h):
            s = (q[bi, :, hi] @ kk[bi, :, hi].T) * scale
            s = np.where(mask[bi] == 0, np.float32(-1e10), s)
            s = s - s.max(axis=-1, keepdims=True)
            e = np.exp(s)
            p = e / e.sum(axis=-1, keepdims=True)
            yb[:, hi] = p @ v[bi, :, hi]
        out[bi] = yb.reshape(t, emb) @ Wo.T + bo
    return out


def kernel(x, mask, Wq, bq, Wk, bk, Wv, bv, Wo, bo):
    global _CACHED_NC, LAST_RESULTS
    x = np.asarray(x, dtype=np.float32)
    mask = np.asarray(mask)
    Wq, Wk, Wv, Wo = (np.asarray(w, dtype=np.float32) for w in (Wq, Wk, Wv, Wo))
    bq, bk, bv, bo = (np.asarray(v_, dtype=np.float32) for v_ in (bq, bk, bv, bo))

    # The device program hardcodes a causal mask and zero q/k/v biases
    # (which is what reference.setup_inputs produces).  Anything else falls
    # back to a plain numpy implementation.
    tril = np.tril(np.ones((T, T), dtype=mask.dtype))
    if (
        x.shape != (B, T, EMB)
        or any(np.any(bias) for bias in (bq, bk, bv))
        or not all(np.array_equal(np.asarray(mask[b_]), tril) for b_ in range(B))
    ):
        return _numpy_fallback(x, mask, Wq, bq, Wk, bk, Wv, bv, Wo, bo)

    from concourse import bass_utils

    f16 = np.float16
    xT = [np.ascontiguousarray(x[b_].T).astype(f16) for b_ in range(B)]
    cmask = _causal_mask_tiles()
    in_maps = []
    for c in range(NCORES):
        b_, hg = c // 2, c % 2
        r = slice(hg * DPC, (hg + 1) * DPC)
        in_maps.append({
            "xT": xT[b_],
            "wqT": np.ascontiguousarray(Wq[r, :].T).astype(f16),
            "wkT": np.ascontiguousarray(Wk[r, :].T).astype(f16),
            "wvT": np.ascontiguousarray(Wv[r, :].T).astype(f16),
            "woT": np.ascontiguousarray(Wo[:, r].T).astype(f16),
            "cmask": cmask,
        })

    if _CACHED_NC is None:
        _CACHED_NC = _build_nc()

    import os
    trace = bool(int(os.environ.get("KERNEL_TRACE", "0")))
    res = bass_utils.run_bass_kernel_spmd(
        _CACHED_NC,
        in_maps,
        core_ids=list(range(NCORES)),
        trace=trace,
    )
    LAST_RESULTS = res
    outs = [np.asarray(r["out"], dtype=np.float32) for r in res.results]
    y = np.stack([outs[2 * b_] + outs[2 * b_ + 1] for b_ in range(B)])
    y += bo[None, None, :]
    return np.ascontiguousarray(y.astype(np.float32))



# revision 47
# speedup vs baseline: 1.1815x; 1.1815x over previous
"""Multi-head self-attention Trainium2 Bass kernel.

Problem: B=4, T=2048, EMB=1024, H=16 heads (head_dim 64), causal mask,
scores scaled by 1/sqrt(EMB), torch-Linear style projections.

Sharding (8 cores): data-parallel over the 4 batches x tensor-parallel over
2 head-groups of 8 heads.  Core c handles batch c//2, heads (c%2)*8..+8.
Each core computes q/k/v projections for its head shard, full TxT causal
attention for its 8 heads, and a partial output projection (its 512 rows of
the unify matmul).  Host sums the two partial outputs per batch and adds bo.

Device layout notes:
 - All PE operands are fp16 (1 col/cycle on the PE); PSUM accumulates fp32.
 - x and the weights are pre-transposed/cast on host so that every matmul
   contraction dim lands on the SBUF partition axis.
 - Scores are computed transposed (pT[s, t] = exp(q.k/32)) so that the
   attn @ v contraction (over s) needs no on-device transposes.  The two
   heads of a head-pair run as adjacent matmuls on disjoint PE row groups
   (contraction rows 0-63 / 64-127) writing the two banks of one [128,1024]
   PSUM tile, so they stream concurrently.
 - Causal column restriction: for the diagonal s-chunk at offset di the
   score/exp/AV work only covers query columns >= 128*di; only the leading
   [128,128] triangle block needs a mask multiply.
 - Softmax sums ride as a 65th "ones" column of v; normalization is a
   fast-approx reciprocal + gpsimd partition broadcast + in-place multiply.
 - Scheduling: the PE executes strictly in-order and its clock ramps only
   under continuous activity (gaps reset the ramp), so (a) x loads are
   split by column-block so block-0 projections start after ~2 MB of DMA,
   (b) warm-up matmuls are strung on DMA arrivals in arrival order, (c)
   each block's pair-2/3 q/k projections and the previous blocks' output
   projections are paced into the ACT-heavy chunk loops as fillers, and
   (d) each pair's final AV + softmax-normalization chain is carried past
   the next pair's first score/exp so the PE never drains at pair/block
   boundaries.
 - The output is stored f16 (halves store traffic); the host upcasts and
   sums the two head-group partials in f32.
"""

import numpy as np

B, T, EMB, H = 4, 2048, 1024, 16
HD = 64          # head dim
HPC = 8          # heads per core
DPC = HPC * HD   # projected dim per core = 512
NCORES = 8
E8 = EMB // 128  # contraction chunks over emb = 8
DP = DPC // 128  # head-pair chunks = 4
TB = T // 512    # t-blocks = 4
SC = T // 128    # s-chunks = 16
VW = HD + 1      # v columns per head incl. ones column = 65

_CACHED_NC = None
LAST_RESULTS = None  # BassKernelResults of the most recent run (for test.py)


def _build_nc():
    import concourse.bacc as bacc
    import concourse.tile as tile
    import concourse.mybir as mybir

    f16 = mybir.dt.float16
    f32 = mybir.dt.float32
    Exp = mybir.ActivationFunctionType.Exp

    nc = bacc.Bacc(
        "TRN2",
        target_bir_lowering=False,
        debug=False,
        enable_asserts=False,
        num_devices=NCORES,
    )

    xT_d = nc.dram_tensor("xT", [EMB, T], f16, kind="ExternalInput").ap()
    wqT_d = nc.dram_tensor("wqT", [EMB, DPC], f16, kind="ExternalInput").ap()
    wkT_d = nc.dram_tensor("wkT", [EMB, DPC], f16, kind="ExternalInput").ap()
    wvT_d = nc.dram_tensor("wvT", [EMB, DPC], f16, kind="ExternalInput").ap()
    woT_d = nc.dram_tensor("woT", [DPC, EMB], f16, kind="ExternalInput").ap()
    cm_d = nc.dram_tensor("cmask", [128, 256], f16, kind="ExternalInput").ap()
    # f16 output: halves the store traffic and the final-DMA tail; the host
    # upcasts and sums the two head-group partials in f32.  The f16
    # rounding adds ~5e-4 relative error against a 2e-2 budget.
    out_d = nc.dram_tensor("out", [T, EMB], f16, kind="ExternalOutput").ap()

    with tile.TileContext(nc) as tc:
        # ---- persistent SBUF tensors (static allocations) -------------
        def sb(name, shape, dt=f16):
            return nc.alloc_sbuf_tensor(name, list(shape), dt).ap()

        xt = [sb(f"xt{k}", [128, T]) for k in range(E8)]
        wq = [sb(f"wq{k}", [128, DPC]) for k in range(E8)]
        wk = [sb(f"wk{k}", [128, DPC]) for k in range(E8)]
        wv = [sb(f"wv{k}", [128, DPC]) for k in range(E8)]
        wo = [sb(f"wo{p}", [128, EMB]) for p in range(DP)]
        cm = sb("cm", [128, 256])
        qt = [sb(f"qt{p}", [128, T]) for p in range(DP)]
        kt = [sb(f"kt{p}", [128, T]) for p in range(DP)]
        vt = sb("vt", [128, SC * HPC * VW])
        ytn = [sb(f"ytn{p}", [128, T]) for p in range(DP)]
        # per-head reciprocal softmax sums for the current t-block, packed
        # on partition 0 (custom DVE ops and partition_broadcast want
        # partition-0-based APs); reused across blocks
        rec = sb("rec", [1, HPC * 512], f32)
        sums = sb("sums", [1, HPC * 512], f32)
        # never initialized: warm-up matmuls read garbage (discarded), so
        # they have no dependencies and can start immediately
        warmsrc = sb("warmsrc", [128, 512])
        # block-3 outproj partials (p=0..2), evacuated here so the final
        # tail only runs the p=3 matmul + fused add+store
        opscr = sb("opscr", [128, 8 * 512], f32)
        # fence targets: tiny SBUF->SBUF DMAs reading the last phase-1 tile
        # keep the phase-2 loads out of the DMA rings until phase-1 lands
        # (the rings fair-share bandwidth among everything in flight, so
        # un-fenced phase-2 stretches the phase-1 ramp ~2x)
        fsc = sb("fsc", [1, 8])

        with (
            tc.tile_pool(name="pp", bufs=2, space="PSUM") as pp,
            tc.tile_pool(name="scp", bufs=2, space="PSUM") as scp,
            tc.tile_pool(name="ytp", bufs=1, space="PSUM") as ytp,
            tc.tile_pool(name="ptp", bufs=8) as ptp,
            tc.tile_pool(name="brecp", bufs=4) as brecp,
            tc.tile_pool(name="ost", bufs=3) as ost,
        ):
            # ---- input loads (direct DMA, alternating between the two
            # HWDGE queue engines; Bacc legalizes multi-dep matmul waits) -
            load_rr = [0]

            def load(dst, src):
                eng = nc.sync if load_rr[0] % 2 == 0 else nc.scalar
                load_rr[0] += 1
                eng.dma_start(dst, src)

            # ones columns for the softmax-sum trick: memset only the 65th
            # column of each head block (the v columns get overwritten by
            # the projection evacuations anyway)
            ones3 = vt[:, :].rearrange("p (x c) -> p x c", c=VW)[:, :, HD:VW]
            nc.vector.memset(ones3, 1.0)
            # PE warm-up: keeps the HAM activity window busy through the
            # whole DMA ramp so real matmuls start at the 2.4 GHz clock.
            # The first burst reads uninitialized SBUF (no deps, starts
            # immediately); later warm-ups read each freshly-DMA'd xt
            # chunk, which strings them out across the load timeline.
            warm = pp.tile([128, 512], f32, tag="pp", name="warmup")

            def warmup_burst(src, n):
                for _ in range(n):
                    nc.tensor.matmul(warm[:, :], src[0:128, 0:128],
                                     src[0:128, 0:512], start=True, stop=True)

            warmup_burst(warmsrc, 8)
            # cm is tiny and gates the first diagonal mask multiply in block
            # 0 -- load it before the big tensors so AV(ck=0) never stalls
            load(cm[:, :], cm_d[:, :])
            # Phase-1 loads: only what the block-0 projections contract over
            # (x columns 0:512 = 1 MB instead of the full 4 MB) plus wq/wv/
            # wk, so block-0 attention starts ~15 us earlier.  The tracker
            # keys dependencies on byte ranges, so consumers of the first
            # 512 columns don't wait for the phase-2 column loads.
            for k in range(E8):
                r = slice(k * 128, (k + 1) * 128)
                load(xt[k][:, 0:512], xT_d[r, 0:512])
                load(wq[k][:, :], wqT_d[r, :])
                warmup_burst(xt[k], 2)
            # wv before wk: the pre units run q0, v0, v1, k0 so the two
            # v-chains amortize the later wk arrival
            for k in range(E8):
                r = slice(k * 128, (k + 1) * 128)
                load(wv[k][:, :], wvT_d[r, :])
            for k in range(E8):
                r = slice(k * 128, (k + 1) * 128)
                load(wk[k][:, :], wkT_d[r, :])
            # DMA fence + ALL phase-2 loads on nc.sync (SP) ONLY.  The
            # fence's RAW wait on wk[7] (the last phase-1 load) stalls the
            # SP sequencer so the phase-2 configs behind it are held back
            # and their transfers don't contend with phase-1 for ring
            # bandwidth.  Critically, nothing phase-2 goes on nc.scalar:
            # that is the Activation engine's sequencer, and a fence or a
            # burst of DMA configs there blocks block-0's exps behind it
            # (the earlier dual-engine second fence measured worse for
            # exactly this reason).  SP has no other duty until the output
            # stores at ~45 us.
            nc.sync.dma_start(fsc[0:1, 0:2], wk[E8 - 1][0:1, 0:2])
            # Phase-2: the remaining x columns (blocks 1-3 projections, run
            # as fillers inside blocks 0-2) and wo (first read ~40 us in).
            # Block-1 columns first so block 0's early(1) fillers (consumed
            # from ~16 us) don't stall on the block-2/3 columns behind them.
            for k in range(E8):
                r = slice(k * 128, (k + 1) * 128)
                nc.sync.dma_start(xt[k][:, 512:1024], xT_d[r, 512:1024])
            for k in range(E8):
                r = slice(k * 128, (k + 1) * 128)
                nc.sync.dma_start(xt[k][:, 1024:T], xT_d[r, 1024:T])
            for p in range(DP):
                nc.sync.dma_start(wo[p][:, :],
                                  woT_d[p * 128:(p + 1) * 128, :])

            def project_tiles(j):
                """Per-tile emit closures for block j's q/k/v projections,
                split into (early, late): early units (q0,q1,k0,k1,v0-3) are
                needed by the block's first head-pairs; late units (q2,k2,
                q3,k3) are only read by pairs 2-3 and are deferred into the
                block's own chunk loop as fillers.  (Evac on DVE -- ACT does
                nothing but exp; gpsimd can't read PSUM.)"""
                tcols = slice(j * 512, (j + 1) * 512)
                units = []

                def qk_unit(w_t, dst, p):
                    def emit():
                        dcols = slice(p * 128, (p + 1) * 128)
                        ps = pp.tile([128, 512], f32, tag="pp",
                                     name=f"ps_{p}_{j}")
                        for e in range(E8):
                            nc.tensor.matmul(
                                ps[:, :], w_t[e][:, dcols], xt[e][:, tcols],
                                start=(e == 0), stop=(e == E8 - 1),
                            )
                        nc.vector.tensor_copy(dst[p][:, tcols], ps[:, :])
                    return emit

                def v_unit(s):
                    def emit():
                        ps = pp.tile([128, 512], f32, tag="pp",
                                     name=f"psv_{s}")
                        for e in range(E8):
                            nc.tensor.matmul(
                                ps[:, :], xt[e][:, s * 128:(s + 1) * 128],
                                wv[e][:, :],
                                start=(e == 0), stop=(e == E8 - 1),
                            )
                        dst = vt[:, s * HPC * VW:(s + 1) * HPC * VW]
                        dst = dst.rearrange("p (h c) -> p h c",
                                            c=VW)[:, :, 0:HD]
                        src = ps[:, :].rearrange("p (h c) -> p h c", c=HD)
                        nc.vector.tensor_copy(dst, src)
                    return emit

                # early: q0,q1 (DMA delivers wq first), then k0,k1, then v
                # chunks; late: pairs 2-3, deferred into the block itself
                units.append(qk_unit(wq, qt, 0))
                units.append(qk_unit(wq, qt, 1))
                units.append(qk_unit(wk, kt, 0))
                units.append(qk_unit(wk, kt, 1))
                for s in range(4 * j, 4 * j + 4):
                    units.append(v_unit(s))
                late = []
                for p in range(2, DP):
                    late.append(qk_unit(wq, qt, p))
                    late.append(qk_unit(wk, kt, p))
                return units, late

            def outproj_tiles(j):
                """Per-tile emit closures for block j's output projection."""
                units = []

                def op_unit(tcn, n):
                    def emit():
                        trows = slice(tcn * 128, (tcn + 1) * 128)
                        ncols = slice(n * 512, (n + 1) * 512)
                        ps = pp.tile([128, 512], f32, tag="pp",
                                     name=f"op_{tcn}_{n}")
                        for p in range(DP):
                            nc.tensor.matmul(
                                ps[:, :], ytn[p][:, trows], wo[p][:, ncols],
                                start=(p == 0), stop=(p == DP - 1),
                            )
                        ot = ost.tile([128, 512], f16, tag="ot",
                                      name=f"ot_{tcn}_{n}")
                        nc.vector.tensor_copy(ot[:, :], ps[:, :])
                        nc.sync.dma_start(out_d[trows, ncols], ot[:, :])
                    return emit

                for tcn in range(4 * j, 4 * j + 4):
                    for n in range(EMB // 512):
                        units.append(op_unit(tcn, n))
                return units

            def outproj_partial_tiles(j):
                """Pass A of block j's output projection: accumulate the
                p=0..2 partials and park them in SBUF scratch.  Emitted as
                tail fillers inside block j's LAST pair (finalize(p=2) is
                carried into that pair's ck1, so these must not emit
                earlier), where the chunk loop is exp-bound and the PE
                would otherwise starve at the throttled late-kernel
                clock."""
                units = []

                def pa_unit(idx, tcn, n):
                    def emit():
                        trows = slice(tcn * 128, (tcn + 1) * 128)
                        ncols = slice(n * 512, (n + 1) * 512)
                        ps = pp.tile([128, 512], f32, tag="pp",
                                     name=f"opa_{tcn}_{n}")
                        for p in range(DP - 1):
                            nc.tensor.matmul(
                                ps[:, :], ytn[p][:, trows], wo[p][:, ncols],
                                start=(p == 0), stop=(p == DP - 2),
                            )
                        nc.vector.tensor_copy(
                            opscr[:, idx * 512:(idx + 1) * 512], ps[:, :])
                    return emit

                idx = 0
                for tcn in range(4 * j, 4 * j + 4):
                    for n in range(EMB // 512):
                        units.append(pa_unit(idx, tcn, n))
                        idx += 1
                return units

            def outproj_close_tiles(j):
                """Pass B (the true tail): only the p=3 matmul, then a
                fused add-partials + f16-cast evacuation and the store."""
                units = []

                def pb_unit(idx, tcn, n):
                    def emit():
                        trows = slice(tcn * 128, (tcn + 1) * 128)
                        ncols = slice(n * 512, (n + 1) * 512)
                        ps = pp.tile([128, 512], f32, tag="pp",
                                     name=f"opb_{tcn}_{n}")
                        nc.tensor.matmul(
                            ps[:, :], ytn[DP - 1][:, trows],
                            wo[DP - 1][:, ncols], start=True, stop=True)
                        ot = ost.tile([128, 512], f16, tag="ot",
                                      name=f"otb_{tcn}_{n}")
                        nc.vector.tensor_add(
                            ot[:, :], ps[:, :],
                            opscr[:, idx * 512:(idx + 1) * 512])
                        nc.sync.dma_start(out_d[trows, ncols], ot[:, :])
                    return emit

                idx = 0
                for tcn in range(4 * j, 4 * j + 4):
                    for n in range(EMB // 512):
                        units.append(pb_unit(idx, tcn, n))
                        idx += 1
                return units

            # carry[0] defers each pair's final AV matmuls (which wait on
            # the pair's last exp+mask) plus its softmax-normalization
            # chain past the NEXT pair's first score/exp emission, so the
            # next pair's PE work hides the exp latency instead of the PE
            # draining at every pair/block boundary (a drain also resets
            # the PE p-state ramp, compounding the cost).
            carry = [None]

            def attend_block(j, fillers, tail_fillers=(), bridge=False):
                """Attention for t-block j.  `fillers` are independent PE
                tile units (next block's projections, previous block's
                output projection) emitted interleaved into the chunk loop
                so the PE always has queued work while ACT runs the exps.
                `tail_fillers` are units that must not emit before the
                last pair's ck1 (they read every pair's normalized ytn);
                they pace over the last pair's remaining chunks."""
                tcols = slice(j * 512, (j + 1) * 512)
                nchunks = 4 * j + 4
                nunits = DP * nchunks
                fill_acc = [0.0]
                fill_step = len(fillers) / nunits
                fillers = list(fillers)
                tail_fillers = list(tail_fillers)
                tail_acc = [0.0]
                tail_step = (len(tail_fillers) / max(1, nchunks - 2)
                             if tail_fillers else 0.0)

                def emit_fill():
                    fill_acc[0] += fill_step
                    while fillers and fill_acc[0] >= 1.0:
                        fillers.pop(0)()
                        fill_acc[0] -= 1.0

                for p in range(DP):
                    yts = [
                        ytp.tile([VW, 512], f32, tag=f"yt{h2}",
                                 name=f"yt{h2}_{p}_{j}")
                        for h2 in range(2)
                    ]
                    # software-pipelined chunk loop: AV for chunk ck is
                    # emitted after the scores+exp of chunk ck+1 so the PE
                    # always has score work while ACT runs the exp.
                    pends = []  # queue of (ck, pt, off, di, last), depth 2

                    def emit_av(ck, pt, off, di, last, yts=yts, p=p):
                        # yts/p bound by default args: the carried call runs
                        # inside the NEXT pair's scope where the loop
                        # variables have been rebound.
                        # One matmul per head covering [off:512]: the mask
                        # multiply of chunk ck finishes on DVE during
                        # exp(ck+1) on ACT, before this AV executes, so no
                        # clean/masked split is needed.  PSUM has_written
                        # bits make the accumulation order safe: the first
                        # executed matmul of the round clears the bank.
                        first = (ck == 0)
                        for h2 in range(2):
                            h = 2 * p + h2
                            vs = vt[:, ck * HPC * VW + h * VW:
                                    ck * HPC * VW + (h + 1) * VW]
                            nc.tensor.matmul(
                                yts[h2][:, off:],
                                vs, pt[:, h2 * 512 + off:(h2 + 1) * 512],
                                start=first, stop=last,
                            )

                    def finalize(yts=yts, p=p):
                        # per h2: sums-copy + recip (gates the broadcast
                        # chain), then the bulk evacuation -- finishing h2=0
                        # entirely before touching h2=1 releases its PSUM
                        # bank sooner
                        for h2 in range(2):
                            base = h2 * 64
                            r = 2 * p + h2
                            rcols = slice(r * 512, (r + 1) * 512)
                            nc.vector.tensor_copy(sums[0:1, rcols],
                                                  yts[h2][HD:HD + 1, :])
                            nc.vector.reciprocal_approx_fast(
                                rec[0:1, rcols], sums[0:1, rcols])
                            nc.vector.tensor_copy(
                                ytn[p][base:base + 64, tcols],
                                yts[h2][0:HD, :])
                        # deferred normalization for this (p, block)
                        for h2 in range(2):
                            base = h2 * 64
                            r = 2 * p + h2
                            rcols = slice(r * 512, (r + 1) * 512)
                            brec = brecp.tile([128, 512], f32, tag="brec",
                                              name=f"brec{h2}_{p}_{j}")
                            nc.gpsimd.partition_broadcast(brec[:, :],
                                                          rec[0:1, rcols])
                            nc.vector.tensor_mul(
                                ytn[p][base:base + 64, tcols],
                                ytn[p][base:base + 64, tcols],
                                brec[base:base + 64, :],
                            )

                    for ck in range(nchunks):
                        di = ck - 4 * j
                        off = 128 * di if di > 0 else 0
                        sc = scp.tile([128, 1024], f32, tag="sc",
                                      name=f"sc_{p}_{j}_{ck}")
                        pt = ptp.tile([128, 1024], f16, tag="pt",
                                      name=f"pt_{p}_{j}_{ck}")
                        # the two heads on disjoint PE row groups (0-63 /
                        # 64-127) target the two banks of sc -> concurrent
                        for h2 in range(2):
                            base = h2 * 64
                            nc.tensor.matmul(
                                sc[:, h2 * 512 + off:(h2 + 1) * 512],
                                kt[p][base:base + 64,
                                      ck * 128:(ck + 1) * 128],
                                qt[p][base:base + 64,
                                      j * 512 + off:(j + 1) * 512],
                                start=True, stop=True,
                            )
                        # one exp for both heads (strided 3D AP over the
                        # valid column ranges)
                        sc3 = sc[:, :].rearrange("p (h c) -> p h c",
                                                 c=512)[:, :, off:]
                        pt3 = pt[:, :].rearrange("p (h c) -> p h c",
                                                 c=512)[:, :, off:]
                        nc.scalar.activation(pt3, sc3, Exp, scale=1.0 / 32.0)
                        if di >= 0:
                            # triangle mask on the leading 128 columns of
                            # both heads' valid ranges.  Two flat 2D
                            # multiplies instead of one strided 3D AP: all-
                            # SBUF packed 16-bit operands qualify for the
                            # DVE 4x perf mode, which the 3D form does not.
                            # NOTE: must stay on DVE -- gpsimd executes this
                            # multiply ~10x slower (measured 505 us vs 298
                            # us whole-kernel)
                            for h2 in range(2):
                                c0 = h2 * 512 + off
                                nc.vector.tensor_mul(
                                    pt[:, c0:c0 + 128], pt[:, c0:c0 + 128],
                                    cm[:, 0:128])
                        # NOTE: a 2-deep AV pipeline (AV(ck-2) after
                        # exp(ck)) measured worse -- the 2-buf sc pool just
                        # moves the stall from AV to scores(ck+2), and the
                        # reshuffled transitions added ~2 us of gaps
                        if pends:
                            if carry[0] is not None:
                                # previous pair's deferred final AV +
                                # softmax normalization: flushed at ck1,
                                # after two of this pair's score/exp
                                # emissions, maximizing cover of the
                                # previous pair's last exp+mask latency
                                # (must precede this pair's first AV, which
                                # re-opens the aliased yts accumulators)
                                carry[0]()
                                carry[0] = None
                            emit_av(*pends.pop(0))
                        emit_fill()
                        if p == DP - 1 and ck >= nchunks - 2 and not fillers:
                            # terminal-drain cover: the block's last AVs
                            # wait on the final exp+mask with no fillers
                            # and no next-pair scores left; a warmup here
                            # keeps the PE busy and the clock ramped
                            warmup_burst(warmsrc, 1)
                        if p == DP - 1 and ck >= 2 and tail_fillers:
                            tail_acc[0] += tail_step
                            while tail_fillers and tail_acc[0] >= 1.0 - 1e-9:
                                tail_fillers.pop(0)()
                                tail_acc[0] -= 1.0
                        pends.append((ck, pt, off, di, ck == nchunks - 1))

                    # one AV still pending: carry it (and this pair's
                    # normalization) past the next pair's first score/exps
                    last_pt = pends[0][1]

                    def make_carry(emit_av=emit_av, pend=pends[0],
                                   fin=finalize):
                        def c():
                            emit_av(*pend)
                            fin()
                        return c
                    carry[0] = make_carry()
                    if p == DP - 1:
                        # flush any fillers left over from float pacing --
                        # fill_step accumulation can round below len(fillers)
                        while fillers:
                            fillers.pop(0)()
                        while tail_fillers:
                            tail_fillers.pop(0)()
                    if bridge and p == DP - 1:
                        carry[0]()
                        carry[0] = None
                        # dummy matmuls reading the final pt tile bridge the
                        # last normalization chain so HAM stays at full
                        # clock for the final output projection; the pt
                        # dependency stops the static scheduler from
                        # hoisting them earlier
                        lpt = last_pt
                        warm2 = pp.tile([128, 512], f32, tag="pp",
                                        name=f"wbr_{j}")
                        for _ in range(22):
                            nc.tensor.matmul(warm2[:, :], lpt[0:128, 0:128],
                                             lpt[0:128, 0:512],
                                             start=True, stop=True)

            # Filler rebalance: block 3 has the most exp work (16 of 40
            # s-chunks) but, in the naive schedule, the fewest fillers, so
            # it runs ACT-bound while blocks 0-2 are PE-bound.  Defer each
            # block's pair-2/3 q/k projections into its own chunk loop
            # (late units, ordered first so they finish before pair 2
            # starts) and push outproj(1) from block 2 into block 3.
            early = {}
            late = {}
            for j in range(TB):
                early[j], late[j] = project_tiles(j)
            # Minimal serial pre-phase: q0, v0, v1, k0 (matching the DMA
            # arrival order wq -> wv -> wk); q1/v2/v3/k1 move into block 0's
            # fillers, paced one per chunk-unit so each lands just before
            # its first consumer (k1 by pair 1, v2/v3 by AV chunks 2/3).
            # Warm-ups between the pre units, gated on progressively later
            # DMA arrivals (wv lands during q0, wk during v0/v1), so
            # nothing later-gated ever queues ahead of ready work on the
            # in-order PE
            e0 = early[0]
            e0[0]()                       # q0 (needs xt cols 0:512 + wq)
            for k in range(E8):
                warmup_burst(wv[k], 1)
            e0[4]()                       # v0 (needs wv)
            e0[5]()                       # v1
            e0[2]()                       # k0 (needs wk)
            block_fill = {
                0: [e0[1], e0[6], e0[7], e0[3]] + late[0] + early[1],
                1: late[1] + early[2] + outproj_tiles(0),
                2: late[2] + early[3],
                3: late[3] + outproj_tiles(1) + outproj_tiles(2),
            }
            # NOTE: outproj_partial/close_tiles (split tail) measured
            # structurally WORSE: block 3 is PE-bound at the throttled
            # late-kernel clock, so the pass-A fillers extend it 1:1 while
            # the DVE-serialized pass-B tail adds ~2 us.  Keep the plain
            # 4-matmul tail.
            for j in range(TB):
                attend_block(j, block_fill[j], bridge=(j == TB - 1))
            for u in outproj_tiles(TB - 1):
                u()

    nc.compile()
    return nc


def _causal_mask_tiles() -> np.ndarray:
    """[128, 256] fp16: the 128x128 causal triangle (keep key p <= query c)
    stored twice side by side so a [128, 2, 128] strided AP lines up with
    the two heads' column blocks of a pt tile."""
    p = np.arange(128)[:, None]
    c = np.arange(128)[None, :]
    tri = (p <= c).astype(np.float16)
    return np.ascontiguousarray(np.concatenate([tri, tri], axis=1))


def _numpy_fallback(x, mask, Wq, bq, Wk, bk, Wv, bv, Wo, bo):
    b, t, emb = x.shape
    h = H
    k = emb // h
    q = (x @ Wq.T + bq).reshape(b, t, h, k)
    kk = (x @ Wk.T + bk).reshape(b, t, h, k)
    v = (x @ Wv.T + bv).reshape(b, t, h, k)
    scale = 1.0 / np.sqrt(emb)
    out = np.empty((b, t, emb), dtype=np.float32)
    for bi in range(b):
        yb = np.empty((t, h, k), dtype=np.float32)
        for hi in range(h):
            s = (q[bi, :, hi] @ kk[bi, :, hi].T) * scale
            s = np.where(mask[bi] == 0, np.float32(-1e10), s)
            s = s - s.max(axis=-1, keepdims=True)
            e = np.exp(s)
            p = e / e.sum(axis=-1, keepdims=True)
            yb[:, hi] = p @ v[bi, :, hi]
        out[bi] = yb.reshape(t, emb) @ Wo.T + bo
    return out


def kernel(x, mask, Wq, bq, Wk, bk, Wv, bv, Wo, bo):
    global _CACHED_NC, LAST_RESULTS
    x = np.asarray(x, dtype=np.float32)
    mask = np.asarray(mask)
    Wq, Wk, Wv, Wo = (np.asarray(w, dtype=np.float32) for w in (Wq, Wk, Wv, Wo))
    bq, bk, bv, bo = (np.asarray(v_, dtype=np.float32) for v_ in (bq, bk, bv, bo))

    # The device program hardcodes a causal mask and zero q/k/v biases
    # (which is what reference.setup_inputs produces).  Anything else falls
    # back to a plain numpy implementation.
    tril = np.tril(np.ones((T, T), dtype=mask.dtype))
    if (
        x.shape != (B, T, EMB)
        or any(np.any(bias) for bias in (bq, bk, bv))
        or not all(np.array_equal(np.asarray(mask[b_]), tril) for b_ in range(B))
    ):
        return _numpy_fallback(x, mask, Wq, bq, Wk, bk, Wv, bv, Wo, bo)

    from concourse import bass_utils

    f16 = np.float16
    xT = [np.ascontiguousarray(x[b_].T).astype(f16) for b_ in range(B)]
    cmask = _causal_mask_tiles()
    in_maps = []
    for c in range(NCORES):
        b_, hg = c // 2, c % 2
        r = slice(hg * DPC, (hg + 1) * DPC)
        in_maps.append({
            "xT": xT[b_],
            "wqT": np.ascontiguousarray(Wq[r, :].T).astype(f16),
            "wkT": np.ascontiguousarray(Wk[r, :].T).astype(f16),
            "wvT": np.ascontiguousarray(Wv[r, :].T).astype(f16),
            "woT": np.ascontiguousarray(Wo[:, r].T).astype(f16),
            "cmask": cmask,
        })

    if _CACHED_NC is None:
        _CACHED_NC = _build_nc()

    import os
    trace = bool(int(os.environ.get("KERNEL_TRACE", "0")))
    res = bass_utils.run_bass_kernel_spmd(
        _CACHED_NC,
        in_maps,
        core_ids=list(range(NCORES)),
        trace=trace,
    )
    LAST_RESULTS = res
    outs = [np.asarray(r["out"], dtype=np.float32) for r in res.results]
    y = np.stack([outs[2 * b_] + outs[2 * b_ + 1] for b_ in range(B)])
    y += bo[None, None, :]
    return np.ascontiguousarray(y.astype(np.float32))



# revision 48
# speedup vs baseline: 1.1860x; 1.0038x over previous
"""Multi-head self-attention Trainium2 Bass kernel.

Problem: B=4, T=2048, EMB=1024, H=16 heads (head_dim 64), causal mask,
scores scaled by 1/sqrt(EMB), torch-Linear style projections.

Sharding (8 cores): data-parallel over the 4 batches x tensor-parallel over
2 head-groups of 8 heads.  Core c handles batch c//2, heads (c%2)*8..+8.
Each core computes q/k/v projections for its head shard, full TxT causal
attention for its 8 heads, and a partial output projection (its 512 rows of
the unify matmul).  Host sums the two partial outputs per batch and adds bo.

Device layout notes:
 - All PE operands are fp16 (1 col/cycle on the PE); PSUM accumulates fp32.
 - x and the weights are pre-transposed/cast on host so that every matmul
   contraction dim lands on the SBUF partition axis.
 - Scores are computed transposed (pT[s, t] = exp(q.k/32)) so that the
   attn @ v contraction (over s) needs no on-device transposes.  The two
   heads of a head-pair run as adjacent matmuls on disjoint PE row groups
   (contraction rows 0-63 / 64-127) writing the two banks of one [128,1024]
   PSUM tile, so they stream concurrently.
 - Causal column restriction: for the diagonal s-chunk at offset di the
   score/exp/AV work only covers query columns >= 128*di; only the leading
   [128,128] triangle block needs a mask multiply.
 - Softmax sums ride as a 65th "ones" column of v; normalization is a
   fast-approx reciprocal + gpsimd partition broadcast + in-place multiply.
 - Scheduling: the PE executes strictly in-order and its clock ramps only
   under continuous activity (gaps reset the ramp), so (a) x loads are
   split by column-block so block-0 projections start after ~2 MB of DMA,
   (b) warm-up matmuls are strung on DMA arrivals in arrival order, (c)
   each block's pair-2/3 q/k projections and the previous blocks' output
   projections are paced into the ACT-heavy chunk loops as fillers, and
   (d) each pair's final AV + softmax-normalization chain is carried past
   the next pair's first score/exp so the PE never drains at pair/block
   boundaries.
 - The output is stored f16 (halves store traffic); the host upcasts and
   sums the two head-group partials in f32.
"""

import numpy as np

B, T, EMB, H = 4, 2048, 1024, 16
HD = 64          # head dim
HPC = 8          # heads per core
DPC = HPC * HD   # projected dim per core = 512
NCORES = 8
E8 = EMB // 128  # contraction chunks over emb = 8
DP = DPC // 128  # head-pair chunks = 4
TB = T // 512    # t-blocks = 4
SC = T // 128    # s-chunks = 16
VW = HD + 1      # v columns per head incl. ones column = 65

_CACHED_NC = None
LAST_RESULTS = None  # BassKernelResults of the most recent run (for test.py)


def _build_nc():
    import concourse.bacc as bacc
    import concourse.tile as tile
    import concourse.mybir as mybir

    f16 = mybir.dt.float16
    f32 = mybir.dt.float32
    Exp = mybir.ActivationFunctionType.Exp

    nc = bacc.Bacc(
        "TRN2",
        target_bir_lowering=False,
        debug=False,
        enable_asserts=False,
        num_devices=NCORES,
    )

    xT_d = nc.dram_tensor("xT", [EMB, T], f16, kind="ExternalInput").ap()
    wqT_d = nc.dram_tensor("wqT", [EMB, DPC], f16, kind="ExternalInput").ap()
    wkT_d = nc.dram_tensor("wkT", [EMB, DPC], f16, kind="ExternalInput").ap()
    wvT_d = nc.dram_tensor("wvT", [EMB, DPC], f16, kind="ExternalInput").ap()
    woT_d = nc.dram_tensor("woT", [DPC, EMB], f16, kind="ExternalInput").ap()
    cm_d = nc.dram_tensor("cmask", [128, 256], f16, kind="ExternalInput").ap()
    # f16 output: halves the store traffic and the final-DMA tail; the host
    # upcasts and sums the two head-group partials in f32.  The f16
    # rounding adds ~5e-4 relative error against a 2e-2 budget.
    out_d = nc.dram_tensor("out", [T, EMB], f16, kind="ExternalOutput").ap()

    with tile.TileContext(nc) as tc:
        # ---- persistent SBUF tensors (static allocations) -------------
        def sb(name, shape, dt=f16):
            return nc.alloc_sbuf_tensor(name, list(shape), dt).ap()

        xt = [sb(f"xt{k}", [128, T]) for k in range(E8)]
        wq = [sb(f"wq{k}", [128, DPC]) for k in range(E8)]
        wk = [sb(f"wk{k}", [128, DPC]) for k in range(E8)]
        wv = [sb(f"wv{k}", [128, DPC]) for k in range(E8)]
        wo = [sb(f"wo{p}", [128, EMB]) for p in range(DP)]
        cm = sb("cm", [128, 256])
        qt = [sb(f"qt{p}", [128, T]) for p in range(DP)]
        kt = [sb(f"kt{p}", [128, T]) for p in range(DP)]
        vt = sb("vt", [128, SC * HPC * VW])
        ytn = [sb(f"ytn{p}", [128, T]) for p in range(DP)]
        # per-head reciprocal softmax sums for the current t-block, packed
        # on partition 0 (custom DVE ops and partition_broadcast want
        # partition-0-based APs); reused across blocks
        rec = sb("rec", [1, HPC * 512], f32)
        sums = sb("sums", [1, HPC * 512], f32)
        # never initialized: warm-up matmuls read garbage (discarded), so
        # they have no dependencies and can start immediately
        warmsrc = sb("warmsrc", [128, 512])
        # block-3 outproj partials (p=0..2), evacuated here so the final
        # tail only runs the p=3 matmul + fused add+store
        opscr = sb("opscr", [128, 8 * 512], f32)
        # fence targets: tiny SBUF->SBUF DMAs reading the last phase-1 tile
        # keep the phase-2 loads out of the DMA rings until phase-1 lands
        # (the rings fair-share bandwidth among everything in flight, so
        # un-fenced phase-2 stretches the phase-1 ramp ~2x)
        fsc = sb("fsc", [1, 8])

        with (
            tc.tile_pool(name="pp", bufs=2, space="PSUM") as pp,
            tc.tile_pool(name="scp", bufs=2, space="PSUM") as scp,
            tc.tile_pool(name="ytp", bufs=1, space="PSUM") as ytp,
            tc.tile_pool(name="ptp", bufs=8) as ptp,
            tc.tile_pool(name="brecp", bufs=4) as brecp,
            tc.tile_pool(name="ost", bufs=3) as ost,
        ):
            # ---- input loads (direct DMA, alternating between the two
            # HWDGE queue engines; Bacc legalizes multi-dep matmul waits) -
            load_rr = [0]

            def load(dst, src):
                eng = nc.sync if load_rr[0] % 2 == 0 else nc.scalar
                load_rr[0] += 1
                eng.dma_start(dst, src)

            # ones columns for the softmax-sum trick: memset only the 65th
            # column of each head block (the v columns get overwritten by
            # the projection evacuations anyway)
            ones3 = vt[:, :].rearrange("p (x c) -> p x c", c=VW)[:, :, HD:VW]
            nc.vector.memset(ones3, 1.0)
            # PE warm-up: keeps the HAM activity window busy through the
            # whole DMA ramp so real matmuls start at the 2.4 GHz clock.
            # The first burst reads uninitialized SBUF (no deps, starts
            # immediately); later warm-ups read each freshly-DMA'd xt
            # chunk, which strings them out across the load timeline.
            warm = pp.tile([128, 512], f32, tag="pp", name="warmup")

            def warmup_burst(src, n):
                for _ in range(n):
                    nc.tensor.matmul(warm[:, :], src[0:128, 0:128],
                                     src[0:128, 0:512], start=True, stop=True)

            warmup_burst(warmsrc, 8)
            # cm is tiny and gates the first diagonal mask multiply in block
            # 0 -- load it before the big tensors so AV(ck=0) never stalls
            load(cm[:, :], cm_d[:, :])
            # Phase-1 loads: only what the block-0 projections contract over
            # (x columns 0:512 = 1 MB instead of the full 4 MB) plus wq/wv/
            # wk, so block-0 attention starts ~15 us earlier.  The tracker
            # keys dependencies on byte ranges, so consumers of the first
            # 512 columns don't wait for the phase-2 column loads.
            for k in range(E8):
                r = slice(k * 128, (k + 1) * 128)
                load(xt[k][:, 0:512], xT_d[r, 0:512])
                load(wq[k][:, :], wqT_d[r, :])
                warmup_burst(xt[k], 2)
            # wv before wk: the pre units run q0, v0, v1, k0 so the two
            # v-chains amortize the later wk arrival
            for k in range(E8):
                r = slice(k * 128, (k + 1) * 128)
                load(wv[k][:, :], wvT_d[r, :])
            for k in range(E8):
                r = slice(k * 128, (k + 1) * 128)
                load(wk[k][:, :], wkT_d[r, :])
            # DMA fence + ALL phase-2 loads on nc.sync (SP) ONLY.  The
            # fence's RAW wait on wk[7] (the last phase-1 load) stalls the
            # SP sequencer so the phase-2 configs behind it are held back
            # and their transfers don't contend with phase-1 for ring
            # bandwidth.  Critically, nothing phase-2 goes on nc.scalar:
            # that is the Activation engine's sequencer, and a fence or a
            # burst of DMA configs there blocks block-0's exps behind it
            # (the earlier dual-engine second fence measured worse for
            # exactly this reason).  SP has no other duty until the output
            # stores at ~45 us.
            nc.sync.dma_start(fsc[0:1, 0:2], wk[E8 - 1][0:1, 0:2])
            # Phase-2: the remaining x columns (blocks 1-3 projections, run
            # as fillers inside blocks 0-2) and wo (first read ~40 us in).
            # Block-1 columns first so block 0's early(1) fillers (consumed
            # from ~16 us) don't stall on the block-2/3 columns behind them.
            for k in range(E8):
                r = slice(k * 128, (k + 1) * 128)
                nc.sync.dma_start(xt[k][:, 512:1024], xT_d[r, 512:1024])
            for k in range(E8):
                r = slice(k * 128, (k + 1) * 128)
                nc.sync.dma_start(xt[k][:, 1024:T], xT_d[r, 1024:T])
            for p in range(DP):
                nc.sync.dma_start(wo[p][:, :],
                                  woT_d[p * 128:(p + 1) * 128, :])

            def project_tiles(j):
                """Per-tile emit closures for block j's q/k/v projections,
                split into (early, late): early units (q0,q1,k0,k1,v0-3) are
                needed by the block's first head-pairs; late units (q2,k2,
                q3,k3) are only read by pairs 2-3 and are deferred into the
                block's own chunk loop as fillers.  (Evac on DVE -- ACT does
                nothing but exp; gpsimd can't read PSUM.)"""
                tcols = slice(j * 512, (j + 1) * 512)
                units = []

                def qk_unit(w_t, dst, p):
                    def emit():
                        dcols = slice(p * 128, (p + 1) * 128)
                        ps = pp.tile([128, 512], f32, tag="pp",
                                     name=f"ps_{p}_{j}")
                        for e in range(E8):
                            nc.tensor.matmul(
                                ps[:, :], w_t[e][:, dcols], xt[e][:, tcols],
                                start=(e == 0), stop=(e == E8 - 1),
                            )
                        nc.vector.tensor_copy(dst[p][:, tcols], ps[:, :])
                    return emit

                def v_unit(s):
                    def emit():
                        ps = pp.tile([128, 512], f32, tag="pp",
                                     name=f"psv_{s}")
                        for e in range(E8):
                            nc.tensor.matmul(
                                ps[:, :], xt[e][:, s * 128:(s + 1) * 128],
                                wv[e][:, :],
                                start=(e == 0), stop=(e == E8 - 1),
                            )
                        dst = vt[:, s * HPC * VW:(s + 1) * HPC * VW]
                        dst = dst.rearrange("p (h c) -> p h c",
                                            c=VW)[:, :, 0:HD]
                        src = ps[:, :].rearrange("p (h c) -> p h c", c=HD)
                        nc.vector.tensor_copy(dst, src)
                    return emit

                # early: q0,q1 (DMA delivers wq first), then k0,k1, then v
                # chunks; late: pairs 2-3, deferred into the block itself
                units.append(qk_unit(wq, qt, 0))
                units.append(qk_unit(wq, qt, 1))
                units.append(qk_unit(wk, kt, 0))
                units.append(qk_unit(wk, kt, 1))
                for s in range(4 * j, 4 * j + 4):
                    units.append(v_unit(s))
                late = []
                for p in range(2, DP):
                    late.append(qk_unit(wq, qt, p))
                    late.append(qk_unit(wk, kt, p))
                return units, late

            def outproj_tiles(j):
                """Per-tile emit closures for block j's output projection."""
                units = []

                def op_unit(tcn, n):
                    def emit():
                        trows = slice(tcn * 128, (tcn + 1) * 128)
                        ncols = slice(n * 512, (n + 1) * 512)
                        ps = pp.tile([128, 512], f32, tag="pp",
                                     name=f"op_{tcn}_{n}")
                        for p in range(DP):
                            nc.tensor.matmul(
                                ps[:, :], ytn[p][:, trows], wo[p][:, ncols],
                                start=(p == 0), stop=(p == DP - 1),
                            )
                        ot = ost.tile([128, 512], f16, tag="ot",
                                      name=f"ot_{tcn}_{n}")
                        nc.vector.tensor_copy(ot[:, :], ps[:, :])
                        nc.sync.dma_start(out_d[trows, ncols], ot[:, :])
                    return emit

                for tcn in range(4 * j, 4 * j + 4):
                    for n in range(EMB // 512):
                        units.append(op_unit(tcn, n))
                return units

            def outproj_partial_tiles(j):
                """Pass A of block j's output projection: accumulate the
                p=0..2 partials and park them in SBUF scratch.  Emitted as
                tail fillers inside block j's LAST pair (finalize(p=2) is
                carried into that pair's ck1, so these must not emit
                earlier), where the chunk loop is exp-bound and the PE
                would otherwise starve at the throttled late-kernel
                clock."""
                units = []

                def pa_unit(idx, tcn, n):
                    def emit():
                        trows = slice(tcn * 128, (tcn + 1) * 128)
                        ncols = slice(n * 512, (n + 1) * 512)
                        ps = pp.tile([128, 512], f32, tag="pp",
                                     name=f"opa_{tcn}_{n}")
                        for p in range(DP - 1):
                            nc.tensor.matmul(
                                ps[:, :], ytn[p][:, trows], wo[p][:, ncols],
                                start=(p == 0), stop=(p == DP - 2),
                            )
                        nc.vector.tensor_copy(
                            opscr[:, idx * 512:(idx + 1) * 512], ps[:, :])
                    return emit

                idx = 0
                for tcn in range(4 * j, 4 * j + 4):
                    for n in range(EMB // 512):
                        units.append(pa_unit(idx, tcn, n))
                        idx += 1
                return units

            def outproj_close_tiles(j):
                """Pass B (the true tail): only the p=3 matmul, then a
                fused add-partials + f16-cast evacuation and the store."""
                units = []

                def pb_unit(idx, tcn, n):
                    def emit():
                        trows = slice(tcn * 128, (tcn + 1) * 128)
                        ncols = slice(n * 512, (n + 1) * 512)
                        ps = pp.tile([128, 512], f32, tag="pp",
                                     name=f"opb_{tcn}_{n}")
                        nc.tensor.matmul(
                            ps[:, :], ytn[DP - 1][:, trows],
                            wo[DP - 1][:, ncols], start=True, stop=True)
                        ot = ost.tile([128, 512], f16, tag="ot",
                                      name=f"otb_{tcn}_{n}")
                        nc.vector.tensor_add(
                            ot[:, :], ps[:, :],
                            opscr[:, idx * 512:(idx + 1) * 512])
                        nc.sync.dma_start(out_d[trows, ncols], ot[:, :])
                    return emit

                idx = 0
                for tcn in range(4 * j, 4 * j + 4):
                    for n in range(EMB // 512):
                        units.append(pb_unit(idx, tcn, n))
                        idx += 1
                return units

            # carry[0] defers each pair's final AV matmuls (which wait on
            # the pair's last exp+mask) plus its softmax-normalization
            # chain past the NEXT pair's first score/exp emission, so the
            # next pair's PE work hides the exp latency instead of the PE
            # draining at every pair/block boundary (a drain also resets
            # the PE p-state ramp, compounding the cost).
            carry = [None]

            def attend_block(j, fillers, tail_fillers=(), bridge=False):
                """Attention for t-block j.  `fillers` are independent PE
                tile units (next block's projections, previous block's
                output projection) emitted interleaved into the chunk loop
                so the PE always has queued work while ACT runs the exps.
                `tail_fillers` are units that must not emit before the
                last pair's ck1 (they read every pair's normalized ytn);
                they pace over the last pair's remaining chunks."""
                tcols = slice(j * 512, (j + 1) * 512)
                nchunks = 4 * j + 4
                nunits = DP * nchunks
                fill_acc = [0.0]
                fill_step = len(fillers) / nunits
                fillers = list(fillers)
                tail_fillers = list(tail_fillers)
                tail_acc = [0.0]
                tail_step = (len(tail_fillers) / max(1, nchunks - 2)
                             if tail_fillers else 0.0)

                def emit_fill():
                    fill_acc[0] += fill_step
                    while fillers and fill_acc[0] >= 1.0:
                        fillers.pop(0)()
                        fill_acc[0] -= 1.0

                for p in range(DP):
                    yts = [
                        ytp.tile([VW, 512], f32, tag=f"yt{h2}",
                                 name=f"yt{h2}_{p}_{j}")
                        for h2 in range(2)
                    ]
                    # software-pipelined chunk loop: AV for chunk ck is
                    # emitted after the scores+exp of chunk ck+1 so the PE
                    # always has score work while ACT runs the exp.
                    pends = []  # queue of (ck, pt, off, di, last), depth 2

                    def emit_av(ck, pt, off, di, last, yts=yts, p=p):
                        # yts/p bound by default args: the carried call runs
                        # inside the NEXT pair's scope where the loop
                        # variables have been rebound.
                        # One matmul per head covering [off:512]: the mask
                        # multiply of chunk ck finishes on DVE during
                        # exp(ck+1) on ACT, before this AV executes, so no
                        # clean/masked split is needed.  PSUM has_written
                        # bits make the accumulation order safe: the first
                        # executed matmul of the round clears the bank.
                        first = (ck == 0)
                        for h2 in range(2):
                            h = 2 * p + h2
                            vs = vt[:, ck * HPC * VW + h * VW:
                                    ck * HPC * VW + (h + 1) * VW]
                            nc.tensor.matmul(
                                yts[h2][:, off:],
                                vs, pt[:, h2 * 512 + off:(h2 + 1) * 512],
                                start=first, stop=last,
                            )

                    def finalize(yts=yts, p=p):
                        # per h2: sums-copy + recip (gates the broadcast
                        # chain), then the bulk evacuation -- finishing h2=0
                        # entirely before touching h2=1 releases its PSUM
                        # bank sooner
                        for h2 in range(2):
                            base = h2 * 64
                            r = 2 * p + h2
                            rcols = slice(r * 512, (r + 1) * 512)
                            nc.vector.tensor_copy(sums[0:1, rcols],
                                                  yts[h2][HD:HD + 1, :])
                            nc.vector.reciprocal_approx_fast(
                                rec[0:1, rcols], sums[0:1, rcols])
                            nc.vector.tensor_copy(
                                ytn[p][base:base + 64, tcols],
                                yts[h2][0:HD, :])
                        # deferred normalization for this (p, block)
                        for h2 in range(2):
                            base = h2 * 64
                            r = 2 * p + h2
                            rcols = slice(r * 512, (r + 1) * 512)
                            brec = brecp.tile([128, 512], f32, tag="brec",
                                              name=f"brec{h2}_{p}_{j}")
                            nc.gpsimd.partition_broadcast(brec[:, :],
                                                          rec[0:1, rcols])
                            nc.vector.tensor_mul(
                                ytn[p][base:base + 64, tcols],
                                ytn[p][base:base + 64, tcols],
                                brec[base:base + 64, :],
                            )

                    for ck in range(nchunks):
                        di = ck - 4 * j
                        off = 128 * di if di > 0 else 0
                        sc = scp.tile([128, 1024], f32, tag="sc",
                                      name=f"sc_{p}_{j}_{ck}")
                        pt = ptp.tile([128, 1024], f16, tag="pt",
                                      name=f"pt_{p}_{j}_{ck}")
                        # the two heads on disjoint PE row groups (0-63 /
                        # 64-127) target the two banks of sc -> concurrent
                        for h2 in range(2):
                            base = h2 * 64
                            nc.tensor.matmul(
                                sc[:, h2 * 512 + off:(h2 + 1) * 512],
                                kt[p][base:base + 64,
                                      ck * 128:(ck + 1) * 128],
                                qt[p][base:base + 64,
                                      j * 512 + off:(j + 1) * 512],
                                start=True, stop=True,
                            )
                        # one exp for both heads (strided 3D AP over the
                        # valid column ranges)
                        sc3 = sc[:, :].rearrange("p (h c) -> p h c",
                                                 c=512)[:, :, off:]
                        pt3 = pt[:, :].rearrange("p (h c) -> p h c",
                                                 c=512)[:, :, off:]
                        nc.scalar.activation(pt3, sc3, Exp, scale=1.0 / 32.0)
                        if di >= 0:
                            # triangle mask on the leading 128 columns of
                            # both heads' valid ranges.  Two flat 2D
                            # multiplies instead of one strided 3D AP: all-
                            # SBUF packed 16-bit operands qualify for the
                            # DVE 4x perf mode, which the 3D form does not.
                            # NOTE: must stay on DVE -- gpsimd executes this
                            # multiply ~10x slower (measured 505 us vs 298
                            # us whole-kernel)
                            for h2 in range(2):
                                c0 = h2 * 512 + off
                                nc.vector.tensor_mul(
                                    pt[:, c0:c0 + 128], pt[:, c0:c0 + 128],
                                    cm[:, 0:128])
                        # NOTE: a 2-deep AV pipeline (AV(ck-2) after
                        # exp(ck)) measured worse -- the 2-buf sc pool just
                        # moves the stall from AV to scores(ck+2), and the
                        # reshuffled transitions added ~2 us of gaps
                        if pends:
                            if carry[0] is not None:
                                # previous pair's deferred final AV +
                                # softmax normalization: flushed at ck1,
                                # after two of this pair's score/exp
                                # emissions, maximizing cover of the
                                # previous pair's last exp+mask latency
                                # (must precede this pair's first AV, which
                                # re-opens the aliased yts accumulators)
                                carry[0]()
                                carry[0] = None
                            emit_av(*pends.pop(0))
                        emit_fill()
                        if p == DP - 1 and ck >= 2 and tail_fillers:
                            tail_acc[0] += tail_step
                            while tail_fillers and tail_acc[0] >= 1.0 - 1e-9:
                                tail_fillers.pop(0)()
                                tail_acc[0] -= 1.0
                        pends.append((ck, pt, off, di, ck == nchunks - 1))

                    # one AV still pending: carry it (and this pair's
                    # normalization) past the next pair's first score/exps
                    last_pt = pends[0][1]

                    def make_carry(emit_av=emit_av, pend=pends[0],
                                   fin=finalize):
                        def c():
                            emit_av(*pend)
                            fin()
                        return c
                    carry[0] = make_carry()
                    if p == DP - 1:
                        # flush any fillers left over from float pacing --
                        # fill_step accumulation can round below len(fillers)
                        while fillers:
                            fillers.pop(0)()
                        while tail_fillers:
                            tail_fillers.pop(0)()
                    if bridge and p == DP - 1:
                        carry[0]()
                        carry[0] = None
                        # dummy matmuls reading the final pt tile bridge the
                        # last normalization chain so HAM stays at full
                        # clock for the final output projection; the pt
                        # dependency stops the static scheduler from
                        # hoisting them earlier
                        lpt = last_pt
                        warm2 = pp.tile([128, 512], f32, tag="pp",
                                        name=f"wbr_{j}")
                        for _ in range(22):
                            nc.tensor.matmul(warm2[:, :], lpt[0:128, 0:128],
                                             lpt[0:128, 0:512],
                                             start=True, stop=True)

            # Filler rebalance: block 3 has the most exp work (16 of 40
            # s-chunks) but, in the naive schedule, the fewest fillers, so
            # it runs ACT-bound while blocks 0-2 are PE-bound.  Defer each
            # block's pair-2/3 q/k projections into its own chunk loop
            # (late units, ordered first so they finish before pair 2
            # starts) and push outproj(1) from block 2 into block 3.
            early = {}
            late = {}
            for j in range(TB):
                early[j], late[j] = project_tiles(j)
            # Minimal serial pre-phase: q0, v0, v1, k0 (matching the DMA
            # arrival order wq -> wv -> wk); q1/v2/v3/k1 move into block 0's
            # fillers, paced one per chunk-unit so each lands just before
            # its first consumer (k1 by pair 1, v2/v3 by AV chunks 2/3).
            # Warm-ups between the pre units, gated on progressively later
            # DMA arrivals (wv lands during q0, wk during v0/v1), so
            # nothing later-gated ever queues ahead of ready work on the
            # in-order PE
            e0 = early[0]
            e0[0]()                       # q0 (needs xt cols 0:512 + wq)
            for k in range(E8):
                warmup_burst(wv[k], 1)
            e0[4]()                       # v0 (needs wv)
            e0[5]()                       # v1
            e0[2]()                       # k0 (needs wk)
            block_fill = {
                0: [e0[1], e0[6], e0[7], e0[3]] + late[0] + early[1],
                1: late[1] + early[2] + outproj_tiles(0),
                2: late[2] + early[3],
                3: late[3] + outproj_tiles(1) + outproj_tiles(2),
            }
            # NOTE: outproj_partial/close_tiles (split tail) measured
            # structurally WORSE: block 3 is PE-bound at the throttled
            # late-kernel clock, so the pass-A fillers extend it 1:1 while
            # the DVE-serialized pass-B tail adds ~2 us.  Keep the plain
            # 4-matmul tail.
            for j in range(TB):
                attend_block(j, block_fill[j], bridge=(j == TB - 1))
            for u in outproj_tiles(TB - 1):
                u()

    nc.compile()
    return nc


def _causal_mask_tiles() -> np.ndarray:
    """[128, 256] fp16: the 128x128 causal triangle (keep key p <= query c)
    stored twice side by side so a [128, 2, 128] strided AP lines up with
    the two heads' column blocks of a pt tile."""
    p = np.arange(128)[:, None]
    c = np.arange(128)[None, :]
    tri = (p <= c).astype(np.float16)
    return np.ascontiguousarray(np.concatenate([tri, tri], axis=1))


def _numpy_fallback(x, mask, Wq, bq, Wk, bk, Wv, bv, Wo, bo):
    b, t, emb = x.shape
    h = H
    k = emb // h
    q = (x @ Wq.T + bq).reshape(b, t, h, k)
    kk = (x @ Wk.T + bk).reshape(b, t, h, k)
    v = (x @ Wv.T + bv).reshape(b, t, h, k)
    scale = 1.0 / np.sqrt(emb)
    out = np.empty((b, t, emb), dtype=np.float32)
    for bi in range(b):
        yb = np.empty((t, h, k), dtype=np.float32)
        for hi in range(h):
            s = (q[bi, :, hi] @ kk[bi, :, hi].T) * scale
            s = np.where(mask[bi] == 0, np.float32(-1e10), s)
            s = s - s.max(axis=-1, keepdims=True)
            e = np.exp(s)
            p = e / e.sum(axis=-1, keepdims=True)
            yb[:, hi] = p @ v[bi, :, hi]
        out[bi] = yb.reshape(t, emb) @ Wo.T + bo
    return out


def kernel(x, mask, Wq, bq, Wk, bk, Wv, bv, Wo, bo):
    global _CACHED_NC, LAST_RESULTS
    x = np.asarray(x, dtype=np.float32)
    mask = np.asarray(mask)
    Wq, Wk, Wv, Wo = (np.asarray(w, dtype=np.float32) for w in (Wq, Wk, Wv, Wo))
    bq, bk, bv, bo = (np.asarray(v_, dtype=np.float32) for v_ in (bq, bk, bv, bo))

    # The device program hardcodes a causal mask and zero q/k/v biases
    # (which is what reference.setup_inputs produces).  Anything else falls
    # back to a plain numpy implementation.
    tril = np.tril(np.ones((T, T), dtype=mask.dtype))
    if (
        x.shape != (B, T, EMB)
        or any(np.any(bias) for bias in (bq, bk, bv))
        or not all(np.array_equal(np.asarray(mask[b_]), tril) for b_ in range(B))
    ):
        return _numpy_fallback(x, mask, Wq, bq, Wk, bk, Wv, bv, Wo, bo)

    from concourse import bass_utils

    f16 = np.float16
    xT = [np.ascontiguousarray(x[b_].T).astype(f16) for b_ in range(B)]
    cmask = _causal_mask_tiles()
    in_maps = []
    for c in range(NCORES):
        b_, hg = c // 2, c % 2
        r = slice(hg * DPC, (hg + 1) * DPC)
        in_maps.append({
            "xT": xT[b_],
            "wqT": np.ascontiguousarray(Wq[r, :].T).astype(f16),
            "wkT": np.ascontiguousarray(Wk[r, :].T).astype(f16),
            "wvT": np.ascontiguousarray(Wv[r, :].T).astype(f16),
            "woT": np.ascontiguousarray(Wo[:, r].T).astype(f16),
            "cmask": cmask,
        })

    if _CACHED_NC is None:
        _CACHED_NC = _build_nc()

    import os
    trace = bool(int(os.environ.get("KERNEL_TRACE", "0")))
    res = bass_utils.run_bass_kernel_spmd(
        _CACHED_NC,
        in_maps,
        core_ids=list(range(NCORES)),
        trace=trace,
    )
    LAST_RESULTS = res
    outs = [np.asarray(r["out"], dtype=np.float32) for r in res.results]
    y = np.stack([outs[2 * b_] + outs[2 * b_ + 1] for b_ in range(B)])
    y += bo[None, None, :]
    return np.ascontiguousarray(y.astype(np.float32))

